# revision 4
# baseline (speedup 1.0000x reference)
"""CrossAttentionFusion Trainium2 kernel.

Problem (hardcoded shapes): B=4, C=256, H=W=32 (N=1024), 8 heads, dk=32.
  s_i = gamma_i * x_i;  Q = Wq@s0+bq;  K/V over the 3 streams concat (3N keys)
  attn = softmax(Q^T K / sqrt(dk));  out = Wo @ (attn V) + bo

Sharding: 8 cores = 4 batches x 2 head-groups (4 heads each).  Each core
computes a partial out-projection over its 128 head-channels; host sums the
two partials per batch (plus the bias terms folded on host).

Device-side layout (per core, all matmuls bf16 with fp32 PSUM):
  - Q, K computed as [128ch, n] via weight-transposed matmuls.
  - V computed directly transposed (V_T[k, d]) with a ones column appended so
    the attn@V matmul also produces the softmax denominator l[q] as row 32.
  - Scores computed transposed S_T[k, q] = K_h^T Q_h (row-tiled 2 heads/pass),
    exp on ScalarE (no max subtraction: scores ~ N(0,1)), then attn@V
    accumulated in PSUM col-tiled 2 heads/pass.
  - softmax division deferred: out_unnorm / l via reciprocal + partition
    broadcast (DRAM roundtrip DMA) after the key loop.
"""

import numpy as np
import ml_dtypes

B, C, HH, WW = 4, 256, 32, 32
N = HH * WW            # 1024
NH = 8                 # heads
DK = C // NH           # 32
HG = 2                 # head groups (cores per batch)
HPG = NH // HG         # 4 heads per group
GC = HPG * DK          # 128 channels per group
NKC = 3 * N // 128     # 24 key chunks of 128
BF16 = ml_dtypes.bfloat16

_CACHE = {}


def _build_program():
    import concourse.tile as tile
    import concourse.mybir as mybir
    from concourse import bacc

    f32 = mybir.dt.float32
    bf16 = mybir.dt.bfloat16
    Exp = mybir.ActivationFunctionType.Exp
    MUL = mybir.AluOpType.mult

    nc = bacc.Bacc("TRN2", target_bir_lowering=False, debug=False)

    s_d = [nc.dram_tensor(f"s{j}", [C, N], bf16, kind="ExternalInput").ap()
           for j in range(3)]
    wqt_d = nc.dram_tensor("wqt", [C, GC], bf16, kind="ExternalInput").ap()
    wkt_d = nc.dram_tensor("wkt", [C, GC], bf16, kind="ExternalInput").ap()
    wvt_d = nc.dram_tensor("wvt", [C, GC], bf16, kind="ExternalInput").ap()
    wot_d = nc.dram_tensor("wot", [GC, C], bf16, kind="ExternalInput").ap()
    bq_d = nc.dram_tensor("bq", [GC, 1], f32, kind="ExternalInput").ap()
    out_d = nc.dram_tensor("outp", [2, 128, N], f32, kind="ExternalOutput").ap()

    with tile.TileContext(nc) as tc:
        with tc.tile_pool(name="consts", bufs=1) as consts:
            s_sb = []
            for j in range(3):
                t = consts.tile([128, 2, N], bf16, tag=f"s{j}")
                nc.sync.dma_start(t[:], s_d[j].rearrange("(cc p) n -> p cc n", p=128))
                s_sb.append(t)
            wqt_sb = consts.tile([128, 2, GC], bf16, tag="wqt")
            nc.sync.dma_start(wqt_sb[:], wqt_d.rearrange("(cc p) m -> p cc m", p=128))
            wkt_sb = consts.tile([128, 2, GC], bf16, tag="wkt")
            nc.sync.dma_start(wkt_sb[:], wkt_d.rearrange("(cc p) m -> p cc m", p=128))
            wvt_sb = consts.tile([128, 2, GC], bf16, tag="wvt")
            nc.sync.dma_start(wvt_sb[:], wvt_d.rearrange("(cc p) m -> p cc m", p=128))
            wot_sb = consts.tile([128, 2, 128], bf16, tag="wot")
            nc.sync.dma_start(wot_sb[:], wot_d.rearrange("d (cc c) -> d cc c", cc=2))
            bq_sb = consts.tile([GC, 1], f32, tag="bq")
            nc.sync.dma_start(bq_sb[:], bq_d)

            ones_sb = consts.tile([1, DK], f32, tag="ones")
            nc.vector.memset(ones_sb[:], 1.0)

            q_sb = consts.tile([128, N], bf16, tag="q")
            k_sb = consts.tile([128, 3 * N], bf16, tag="k")
            vt_sb = consts.tile([128, NKC, HPG, DK + 1], bf16, tag="vt")
            nc.vector.memset(vt_sb[:], 1.0)
            onorm_sb = consts.tile([128, N], bf16, tag="onorm")

            # ---- phase A: projections ----
            with tc.tile_pool(name="psA", bufs=1, space="PSUM") as psA:
                # Q (with bias) and K (bias dropped - softmax shift invariant)
                for kind in range(4):  # 0 -> Q, 1..3 -> K streams
                    ps = psA.tile([128, N], f32, tag="qk", bufs=3)
                    wt = wqt_sb if kind == 0 else wkt_sb
                    src = s_sb[0] if kind == 0 else s_sb[kind - 1]
                    for qh in range(2):
                        for cc in range(2):
                            nc.tensor.matmul(
                                ps[:, qh * 512:(qh + 1) * 512],
                                wt[:, cc, :],
                                src[:, cc, qh * 512:(qh + 1) * 512],
                                start=(cc == 0), stop=(cc == 1),
                            )
                    if kind == 0:
                        nc.vector.tensor_scalar_add(q_sb[:], ps[:], bq_sb[:])
                    else:
                        j = kind - 1
                        nc.vector.tensor_copy(k_sb[:, j * N:(j + 1) * N], ps[:])

                # V transposed, per 128-key chunk
                for kc in range(NKC):
                    j, kb = kc // 8, kc % 8
                    ps = psA.tile([128, 128], f32, tag="vtp", bufs=2)
                    for cc in range(2):
                        nc.tensor.matmul(
                            ps[:],
                            s_sb[j][:, cc, kb * 128:(kb + 1) * 128],
                            wvt_sb[:, cc, :],
                            start=(cc == 0), stop=(cc == 1),
                        )
                    nc.vector.tensor_copy(
                        vt_sb[:, kc, :, 0:DK],
                        ps[:].rearrange("p (h d) -> p h d", h=HPG),
                    )

            # ---- phase B: attention ----
            with (
                tc.tile_pool(name="psB", bufs=1, space="PSUM") as psB,
                tc.tile_pool(name="esb", bufs=3) as esb,
                tc.tile_pool(name="nrm", bufs=2) as nrm,
                tc.tile_pool(name="scr", bufs=2, space="DRAM") as scr,
            ):
                for hp in range(HG):
                    ot = psB.tile([128, N], f32, tag="ot", bufs=2)
                    for kc in range(NKC):
                        for qh in range(2):
                            sps = psB.tile([128, 1024], f32, tag="s", bufs=2)
                            for hh in range(2):
                                h = 2 * hp + hh
                                nc.tensor.matmul(
                                    sps[:, hh * 512:(hh + 1) * 512],
                                    k_sb[32 * h:32 * h + 32, kc * 128:(kc + 1) * 128],
                                    q_sb[32 * h:32 * h + 32, qh * 512:(qh + 1) * 512],
                                    start=True, stop=True,
                                    tile_position=(32 * h, 0),
                                )
                            et = esb.tile([128, 1024], bf16, tag="e")
                            nc.scalar.activation(et[:], sps[:], Exp)
                            for hh in range(2):
                                nc.tensor.matmul(
                                    ot[64 * hh:64 * hh + DK + 1,
                                       qh * 512:(qh + 1) * 512],
                                    vt_sb[:, kc, 2 * hp + hh, :],
                                    et[:, hh * 512:(hh + 1) * 512],
                                    start=(kc == 0), stop=(kc == NKC - 1),
                                    tile_position=(0, 64 * hh),
                                    skip_group_check=True,
                                )
                    # normalize: out / l, l in row 32 of each head slot
                    for hh in range(2):
                        h = 2 * hp + hh
                        r = nrm.tile([1, N], f32, tag="r")
                        nc.vector.reciprocal(r[:], ot[64 * hh + DK:64 * hh + DK + 1, :])
                        rd = scr.tile([1, N], f32, tag="rd")
                        nc.sync.dma_start(rd[:], r[:])
                        bc = nrm.tile([DK, N], f32, tag="bc")
                        nc.sync.dma_start(bc[:], rd[:].to_broadcast((DK, N)))
                        nc.vector.tensor_tensor(
                            onorm_sb[32 * h:32 * h + DK, :],
                            ot[64 * hh:64 * hh + DK, :],
                            bc[:],
                            MUL,
                        )

                # ---- phase C: out-projection (partial over this head group) ----
                with tc.tile_pool(name="osb", bufs=2) as osb:
                    for cc in range(2):
                        ps = psB.tile([128, 1024], f32, tag="s", bufs=2)
                        for qh in range(2):
                            nc.tensor.matmul(
                                ps[:, qh * 512:(qh + 1) * 512],
                                wot_sb[:, cc, :],
                                onorm_sb[:, qh * 512:(qh + 1) * 512],
                                start=True, stop=True,
                            )
                        ob = osb.tile([128, N], f32, tag="ob")
                        nc.vector.tensor_copy(ob[:], ps[:])
                        nc.sync.dma_start(out_d[cc], ob[:])

    nc.compile()
    return nc


def _get_program():
    if "nc" not in _CACHE:
        _CACHE["nc"] = _build_program()
    return _CACHE["nc"]


def _prep_in_maps(x0, x1, x2, gamma, gamma_1, gamma_2, Wq, bq, Wk, Wv, Wo):
    scale = 1.0 / np.sqrt(np.float32(DK))
    xs = [
        (np.float32(g) * np.asarray(x, np.float32)).reshape(B, C, N)
        for g, x in ((gamma, x0), (gamma_1, x1), (gamma_2, x2))
    ]
    in_maps = []
    for core in range(8):
        b, g = core // HG, core % HG
        rows = slice(g * GC, (g + 1) * GC)
        m = {
            "s0": xs[0][b].astype(BF16),
            "s1": xs[1][b].astype(BF16),
            "s2": xs[2][b].astype(BF16),
            "wqt": (Wq[rows, :].astype(np.float32) * scale).T.astype(BF16),
            "wkt": Wk[rows, :].astype(np.float32).T.astype(BF16),
            "wvt": Wv[rows, :].astype(np.float32).T.astype(BF16),
            "wot": Wo[:, rows].astype(np.float32).T.astype(BF16),
            "bq": (bq[rows].astype(np.float32) * scale).reshape(GC, 1),
        }
        in_maps.append({k: np.ascontiguousarray(v) for k, v in m.items()})
    return in_maps


def run(inputs, trace=False):
    """Run on the 8 NeuronCores; returns (full_output, BassKernelResults)."""
    from concourse.bass_utils import run_bass_kernel_spmd

    nc = _get_program()
    in_maps = _prep_in_maps(
        inputs["x0"], inputs["x1"], inputs["x2"],
        inputs["gamma"], inputs["gamma_1"], inputs["gamma_2"],
        inputs["Wq"], inputs["bq"], inputs["Wk"], inputs["Wv"], inputs["Wo"],
    )
    res = run_bass_kernel_spmd(nc, in_maps, core_ids=list(range(8)), trace=trace)

    Wo = np.asarray(inputs["Wo"], np.float32)
    bv = np.asarray(inputs["bv"], np.float32)
    bo = np.asarray(inputs["bo"], np.float32)
    bias = Wo @ bv + bo  # V-bias and out-bias folded on host (exact)

    out = np.zeros((B, C, N), np.float32)
    for core in range(8):
        b = core // HG
        out[b] += res.results[core]["outp"].reshape(C, N)
    out += bias[None, :, None]
    return out.reshape(B, C, HH, WW), res


def kernel(**inputs):
    out, _ = run(inputs, trace=False)
    return out
